# revision 21
# baseline (speedup 1.0000x reference)
"""CLIP contrastive loss on 8 Trainium2 NeuronCores (fp8 DoubleRow + symmetry).

Math (reference): with n = 4096, 2n = 8192 rows of L2-normalized features,
  G[i, t] = scale * <img_i, txt_t>          (i, t in [0, 8192))
  CE_img row r (r<n): lse(G[r, :]) - G[r, r]
  CE_txt row t (t<n): lse(G[:, t]) - G[t, t]
  loss = (mean CE_img + mean CE_txt) / 2.

Only three [4096, 4096] blocks of G are needed:
  B1 = G[:n, :n]   -> row-sums of exp (CE_img) AND col-sums of exp (CE_txt)
  B2 = G[:n, n:]   -> row-sums (CE_img)
  B3 = G[n:, :n]   -> computed transposed (txt[:n] @ img[n:].T): row-sums (CE_txt)
The fourth quadrant G[n:, n:] is never used: 25% less matmul+exp vs the
two full [4096, 8192] logits matrices.

Distribution: core c owns rows [c*512, (c+1)*512) of each pass.  Features are
quantized to fp8 e4m3 on host (sqrt(scale) folded into both operands) and the
matmuls run in DoubleRow perf mode (2 fp8 weights per PE cell, 256-deep
contraction per instruction).  Work is pipelined in [128, 2048] PSUM regions
(4 banks, double-buffered): per region 8 DoubleRow matmuls (weights reused
across 4 consecutive matmuls), then one 2048-wide ScalarE exp IN-PLACE on the
PSUM region (psum->psum streams at the full 1.2 GHz rate; psum->sbuf is ~18%
slower) with accum_out row partials.  PE (~2.1us/region) and ScalarE
(~2.1us/region) run neck and neck; DMA stays ahead.

For B1 (pass 1) the DVE accumulates each exp'd PSUM region into a [128, 4096]
bf16 tile; its partition sums (ones-stationary matmuls) give this core's
col-sum partials, which the host adds across cores.  Host computes diagonal
terms exactly in f64 and assembles the loss.
"""

import numpy as np
import ml_dtypes

import concourse.tile as tile
from concourse import bacc, mybir
from concourse.bass_utils import run_bass_kernel_spmd

TWO_N = 8192   # total rows
N = 4096       # CE rows (= num_logits) and block width
D = 512        # embedding dim
C = 8          # cores
R = 512        # rows per core per pass
KC = D // 128  # 128-deep contraction chunks = 4
MB = R // 128  # 128-row blocks per core = 4
HN = 4096      # logits columns per pass
REG = 2048     # PSUM region width (4 banks)
NPART = 25     # row-partial columns (P1: 0-7, P2: 8-15, P3: 16-24)

BF16 = mybir.dt.bfloat16
F32 = mybir.dt.float32
F8 = mybir.dt.float8e4
FP8NP = ml_dtypes.float8_e4m3

_CACHE = {}

# mov DMA chunk widths per half-pass; P1 starts narrow so the PE starts early
P1_HALVES = [[512, 512, 1024], [1024, 1024]]
PW_HALVES = [[1024, 1024], [1024, 1024]]


def _build():
    nc = bacc.Bacc("TRN2", target_bir_lowering=False, debug=False, num_devices=C)

    stat_img = nc.dram_tensor("stat_img", [128, KC, R], F8, kind="ExternalInput").ap()
    stat_txt = nc.dram_tensor("stat_txt", [128, KC, R], F8, kind="ExternalInput").ap()
    mov_t1 = nc.dram_tensor("mov_t1", [128, KC, HN], F8, kind="ExternalInput").ap()
    mov_t2 = nc.dram_tensor("mov_t2", [128, KC, HN], F8, kind="ExternalInput").ap()
    mov_i2 = nc.dram_tensor("mov_i2", [128, KC, HN], F8, kind="ExternalInput").ap()
    out_rows = nc.dram_tensor("out_rows", [128, NPART], F32, kind="ExternalOutput").ap()
    out_acc = nc.dram_tensor("out_acc", [128, HN], BF16, kind="ExternalOutput").ap()

    DR = mybir.MatmulPerfMode.DoubleRow
    EXP = mybir.ActivationFunctionType.Exp

    with tile.TileContext(nc) as tc:
        with (
            tc.tile_pool(name="fix", bufs=1) as fix_pool,
            tc.tile_pool(name="mov", bufs=6) as mov_pool,
            tc.tile_pool(name="psum", bufs=2, space="PSUM") as psum_pool,
        ):
            st_img = fix_pool.tile([128, KC, R], F8, tag="st_img")
            st_txt = fix_pool.tile([128, KC, R], F8, tag="st_txt")
            acc = fix_pool.tile([128, HN], BF16, tag="acc")
            partials = fix_pool.tile([128, NPART], F32, tag="partials")
            warm = fix_pool.tile([128, 512], BF16, tag="warm")

            # on the otherwise-idle GpSimd engine (DVE handles the pass work)
            nc.gpsimd.memset(warm[:], 0.0)

            # stationary blocks ride the scalar HWDGE ring, parallel with the
            # mov chunks on the sync ring
            nc.scalar.dma_start(st_img[:], stat_img[:])
            nc.scalar.dma_start(st_txt[:], stat_txt[:])

            def do_pass(pi, st, movd, halves, pbase):
                off = 0
                for h, widths in enumerate(halves):
                    # stream this half's mov chunks; chunks[] = (tile, start, w)
                    chunks = []
                    for ci, cw in enumerate(widths):
                        mt = mov_pool.tile([128, KC, cw], F8, tag=f"mt{cw}")
                        if pi == 0 and h == 0:
                            # the head is gated on these chunks: spread them
                            # over two parallel DMA rings so the first region
                            # isn't serialized behind one ring's spin-up
                            ring = (nc.sync, nc.gpsimd, nc.gpsimd)[ci % 3]
                        else:
                            ring = nc.sync
                        ring.dma_start(mt[:], movd[:, :, off:off + cw])
                        chunks.append((mt, off, cw))
                        off += cw
                    a = h * REG
                    for m in range(MB):
                        reg = psum_pool.tile([128, REG], F32, tag="reg")
                        if pi == 0 and h == 0 and m == 0:
                            # HAM warm-up INTO region-0's own tile: the WAW
                            # dependency pins these cold matmuls ahead of the
                            # real (DMA-gated) ones, so the clock gate is open
                            # by the time the first mov chunk lands
                            for _ in range(10):
                                nc.tensor.matmul(
                                    reg[:, 0:512], warm[:, 0:128],
                                    warm[:, 0:512], start=True, stop=True,
                                )
                        for kp in range(2):
                            for mt, cs, cw in chunks:
                                for s in range(0, cw, 512):
                                    g = cs + s - a  # col offset within region
                                    nc.tensor.matmul(
                                        reg[:, g:g + 512],
                                        st[:, 2 * kp:2 * kp + 2,
                                           m * 128:(m + 1) * 128],
                                        mt[:, 2 * kp:2 * kp + 2, s:s + 512],
                                        start=(kp == 0),
                                        stop=(kp == 1),
                                        perf_mode=DR,
                                    )
                        col = pbase + m * 2 + h
                        if pi == 0:
                            # P1 exps land in SBUF bf16 (region freed by the
                            # ACT itself); m0 writes acc directly, m1-3 are
                            # DVE-added into acc at the 2x 16-bit rate,
                            # off the critical path
                            if m == 0:
                                dst = acc[:, a:a + REG]
                            else:
                                dst = fix_pool.tile(
                                    [128, REG], BF16, tag="expt",
                                    name=f"expt{h}_{m}", bufs=2,
                                )
                            nc.scalar.activation(
                                dst, reg[:], EXP, bias=0.0,
                                accum_out=partials[:, col:col + 1],
                            )
                            if m > 0:
                                nc.vector.tensor_add(
                                    acc[:, a:a + REG], acc[:, a:a + REG], dst
                                )
                        elif pi == 2 and m == 3 and h == 1:
                            # split the kernel's last exp to shorten the tail
                            for q in range(2):
                                nc.scalar.activation(
                                    reg[:, q * 1024:(q + 1) * 1024],
                                    reg[:, q * 1024:(q + 1) * 1024],
                                    EXP,
                                    bias=0.0,
                                    accum_out=partials[:, col + q:col + q + 1],
                                )
                        else:
                            nc.scalar.activation(
                                reg[:],
                                reg[:],
                                EXP,
                                bias=0.0,
                                accum_out=partials[:, col:col + 1],
                            )

            # P1: img rows x txt[:n]  (B1)
            do_pass(0, st_img, mov_t1, P1_HALVES, 0)
            # ship B1's per-partition exp sums; the host does the final
            # partition+core reduction for the col sums.  Runs on the gpsimd
            # ring during P2, completely off the critical path.
            nc.gpsimd.dma_start(out_acc[:], acc[:])

            # P2: img rows x txt[n:]  (B2)
            do_pass(1, st_img, mov_t2, PW_HALVES, 8)

            # P3: txt rows x img[n:]  (B3 transposed)
            do_pass(2, st_txt, mov_i2, PW_HALVES, 16)

            # issued by the scalar engine itself right after its last
            # accumulator read: no cross-engine hop on the critical tail
            nc.scalar.dma_start(out_rows[:], partials[:])

    nc.compile()
    return nc


def _get_nc():
    if "nc" not in _CACHE:
        _CACHE["nc"] = _build()
    return _CACHE["nc"]


def _prep_inputs(image_features, text_features, logit_scale):
    img = np.asarray(image_features, dtype=np.float32)
    txt = np.asarray(text_features, dtype=np.float32)
    scale = float(np.asarray(logit_scale, dtype=np.float32))
    sf = np.float32(np.sqrt(scale))  # folded into BOTH operands

    qimg = np.asarray(img * sf, dtype=np.float32).astype(FP8NP)
    qtxt = np.asarray(txt * sf, dtype=np.float32).astype(FP8NP)

    def mov_layout(q):
        # [p, k, c] = q[c, k*128 + p]
        a = np.ascontiguousarray(q.T).reshape(KC, 128, HN)
        return np.ascontiguousarray(a.transpose(1, 0, 2))

    def stat_layout(q, c):
        # [p, k, m] = q[c*R + m, k*128 + p]
        a = np.ascontiguousarray(q[c * R:(c + 1) * R].T).reshape(KC, 128, R)
        return np.ascontiguousarray(a.transpose(1, 0, 2))

    mov_t1 = mov_layout(qtxt[:N])
    mov_t2 = mov_layout(qtxt[N:])
    mov_i2 = mov_layout(qimg[N:])
    in_maps = [
        {
            "stat_img": stat_layout(qimg, c),
            "stat_txt": stat_layout(qtxt, c),
            "mov_t1": mov_t1,
            "mov_t2": mov_t2,
            "mov_i2": mov_i2,
        }
        for c in range(C)
    ]
    # diagonal logits (same for both CE terms): scale * <img_r, txt_r>
    diag = scale * np.sum(
        img[:N].astype(np.float64) * txt[:N].astype(np.float64), axis=1
    )
    return in_maps, diag


def _finish(results, diag):
    P = np.stack([results[c]["out_rows"] for c in range(C)]).astype(np.float64)
    # col sums of exp(B1): reduce the per-core [128, 4096] bf16 partial sums
    # over partitions and cores in f64
    colp = (
        np.stack([results[c]["out_acc"] for c in range(C)])
        .astype(np.float64)
        .sum(axis=(0, 1))
    )
    # [C, 128, MB] row sums; partial col layout: pbase + m*2 + half
    s_img = (
        P[:, :, 0:8].reshape(C, 128, MB, 2).sum(-1)
        + P[:, :, 8:16].reshape(C, 128, MB, 2).sum(-1)
    )
    s_txt = np.empty_like(s_img)
    s_txt[:, :, 0:3] = P[:, :, 16:22].reshape(C, 128, 3, 2).sum(-1)
    s_txt[:, :, 3] = P[:, :, 22:25].sum(-1)
    # global row for (c, p, m): c*R + m*128 + p
    rows = (
        np.arange(C)[:, None, None] * R
        + np.arange(MB)[None, None, :] * 128
        + np.arange(128)[None, :, None]
    )
    s_txt = s_txt + colp[rows]
    d = diag[rows]
    ce_img = np.mean(np.log(s_img) - d)
    ce_txt = np.mean(np.log(s_txt) - d)
    return np.float32((ce_img + ce_txt) / 2.0)


def kernel(image_features, text_features, logit_scale):
    nc = _get_nc()
    in_maps, diag = _prep_inputs(image_features, text_features, logit_scale)
    res = run_bass_kernel_spmd(nc, in_maps, list(range(C)))
    return _finish(res.results, diag)


if __name__ == "__main__":
    rng = np.random.default_rng(0)
    img = rng.standard_normal((TWO_N, D), dtype=np.float32)
    txt = rng.standard_normal((TWO_N, D), dtype=np.float32)
    img /= np.linalg.norm(img, axis=-1, keepdims=True)
    txt /= np.linalg.norm(txt, axis=-1, keepdims=True)
    print(kernel(img, txt, np.float32(100.0)))


# revision 25
# speedup vs baseline: 1.0450x; 1.0450x over previous
"""CLIP contrastive loss on 8 Trainium2 NeuronCores (fp8 DoubleRow + symmetry).

Math (reference): with n = 4096, 2n = 8192 rows of L2-normalized features,
  G[i, t] = scale * <img_i, txt_t>          (i, t in [0, 8192))
  CE_img row r (r<n): lse(G[r, :]) - G[r, r]
  CE_txt row t (t<n): lse(G[:, t]) - G[t, t]
  loss = (mean CE_img + mean CE_txt) / 2.

Only three [4096, 4096] blocks of G are needed:
  B1 = G[:n, :n]   -> row-sums of exp (CE_img) AND col-sums of exp (CE_txt)
  B2 = G[:n, n:]   -> row-sums (CE_img)
  B3 = G[n:, :n]   -> computed transposed (txt[:n] @ img[n:].T): row-sums (CE_txt)
The fourth quadrant G[n:, n:] is never used: 25% less matmul+exp vs the
two full [4096, 8192] logits matrices.

Distribution: core c owns rows [c*512, (c+1)*512) of each pass.  Features are
quantized to fp8 e4m3 on host (sqrt(scale) folded into both operands) and the
matmuls run in DoubleRow perf mode (2 fp8 weights per PE cell, 256-deep
contraction per instruction).  Work is pipelined in [128, 2048] PSUM regions
(4 banks, double-buffered): per region 8 DoubleRow matmuls (weights reused
across 4 consecutive matmuls), then one 2048-wide ScalarE exp IN-PLACE on the
PSUM region (psum->psum streams at the full 1.2 GHz rate; psum->sbuf is ~18%
slower) with accum_out row partials.  PE (~2.1us/region) and ScalarE
(~2.1us/region) run neck and neck; DMA stays ahead.

For B1 (pass 1) the DVE accumulates each exp'd PSUM region into a [128, 4096]
bf16 tile; its partition sums (ones-stationary matmuls) give this core's
col-sum partials, which the host adds across cores.  Host computes diagonal
terms exactly in f64 and assembles the loss.
"""

import numpy as np
import ml_dtypes

import concourse.tile as tile
from concourse import bacc, mybir
from concourse.bass_utils import run_bass_kernel_spmd

TWO_N = 8192   # total rows
N = 4096       # CE rows (= num_logits) and block width
D = 512        # embedding dim
C = 8          # cores
R = 512        # rows per core per pass
KC = D // 128  # 128-deep contraction chunks = 4
MB = R // 128  # 128-row blocks per core = 4
HN = 4096      # logits columns per pass
REG = 2048     # PSUM region width (4 banks)
NPART = 25     # row-partial columns (P1: 0-7, P2: 8-15, P3: 16-24)

BF16 = mybir.dt.bfloat16
F32 = mybir.dt.float32
F8 = mybir.dt.float8e4
FP8NP = ml_dtypes.float8_e4m3

_CACHE = {}

# mov DMA chunk widths per half-pass
P1_HALVES = [[1024, 1024], [1024, 1024]]
PW_HALVES = [[1024, 1024], [1024, 1024]]


def _build():
    nc = bacc.Bacc("TRN2", target_bir_lowering=False, debug=False, num_devices=C)

    stat_img = nc.dram_tensor("stat_img", [128, KC, R], F8, kind="ExternalInput").ap()
    stat_txt = nc.dram_tensor("stat_txt", [128, KC, R], F8, kind="ExternalInput").ap()
    mov_t1 = nc.dram_tensor("mov_t1", [128, KC, HN], F8, kind="ExternalInput").ap()
    mov_t2 = nc.dram_tensor("mov_t2", [128, KC, HN], F8, kind="ExternalInput").ap()
    mov_i2 = nc.dram_tensor("mov_i2", [128, KC, HN], F8, kind="ExternalInput").ap()
    out_rows = nc.dram_tensor("out_rows", [128, NPART], F32, kind="ExternalOutput").ap()
    out_acc = nc.dram_tensor("out_acc", [128, HN], BF16, kind="ExternalOutput").ap()

    DR = mybir.MatmulPerfMode.DoubleRow
    EXP = mybir.ActivationFunctionType.Exp

    with tile.TileContext(nc) as tc:
        with (
            tc.tile_pool(name="fix", bufs=1) as fix_pool,
            tc.tile_pool(name="mov", bufs=6) as mov_pool,
            tc.tile_pool(name="psum", bufs=2, space="PSUM") as psum_pool,
        ):
            st_img = fix_pool.tile([128, KC, R], F8, tag="st_img")
            st_txt = fix_pool.tile([128, KC, R], F8, tag="st_txt")
            acc = fix_pool.tile([128, HN], BF16, tag="acc")
            partials = fix_pool.tile([128, NPART], F32, tag="partials")
            warm = fix_pool.tile([128, 512], BF16, tag="warm")

            # on the otherwise-idle GpSimd engine (DVE handles the pass work)
            nc.gpsimd.memset(warm[:], 0.0)

            # st_img rides the scalar HWDGE ring, parallel with the mov
            # chunks on the sync ring; st_txt (needed only by P3) is emitted
            # after P1 so it doesn't delay the head
            nc.scalar.dma_start(st_img[:], stat_img[:])

            def do_pass(pi, st, movd, halves, pbase):
                off = 0
                for h, widths in enumerate(halves):
                    # stream this half's mov chunks; chunks[] = (tile, start, w)
                    chunks = []
                    for ci, cw in enumerate(widths):
                        mt = mov_pool.tile([128, KC, cw], F8, tag=f"mt{cw}")
                        if pi == 0 and h == 0 and ci == 1:
                            # the head is gated on h0's two chunks: the second
                            # rides the scalar ring (parallel with sync) so
                            # region 0 isn't serialized behind one ring
                            ring = nc.scalar
                        else:
                            ring = nc.sync
                        ring.dma_start(mt[:], movd[:, :, off:off + cw])
                        chunks.append((mt, off, cw))
                        off += cw
                    a = h * REG
                    for m in range(MB):
                        reg = psum_pool.tile([128, REG], F32, tag="reg")
                        if pi == 0 and h == 0 and m == 0:
                            # HAM warm-up INTO region-0's own tile: the WAW
                            # dependency pins these cold matmuls ahead of the
                            # real (DMA-gated) ones, so the clock gate is open
                            # by the time the first mov chunk lands
                            for _ in range(10):
                                nc.tensor.matmul(
                                    reg[:, 0:512], warm[:, 0:128],
                                    warm[:, 0:512], start=True, stop=True,
                                )
                        for kp in range(2):
                            for mt, cs, cw in chunks:
                                for s in range(0, cw, 512):
                                    g = cs + s - a  # col offset within region
                                    nc.tensor.matmul(
                                        reg[:, g:g + 512],
                                        st[:, 2 * kp:2 * kp + 2,
                                           m * 128:(m + 1) * 128],
                                        mt[:, 2 * kp:2 * kp + 2, s:s + 512],
                                        start=(kp == 0),
                                        stop=(kp == 1),
                                        perf_mode=DR,
                                    )
                        col = pbase + m * 2 + h
                        if pi == 0:
                            # P1 exps land in SBUF bf16 (region freed by the
                            # ACT itself); m0 writes acc directly, m1-3 are
                            # DVE-added into acc at the 2x 16-bit rate,
                            # off the critical path
                            if m == 0:
                                dst = acc[:, a:a + REG]
                            else:
                                dst = fix_pool.tile(
                                    [128, REG], BF16, tag="expt",
                                    name=f"expt{h}_{m}", bufs=2,
                                )
                            nc.scalar.activation(
                                dst, reg[:], EXP, bias=0.0,
                                accum_out=partials[:, col:col + 1],
                            )
                            if m > 0:
                                nc.vector.tensor_add(
                                    acc[:, a:a + REG], acc[:, a:a + REG], dst
                                )
                        elif pi == 2 and m == 3 and h == 1:
                            # split the kernel's last exp to shorten the tail
                            for q in range(2):
                                nc.scalar.activation(
                                    reg[:, q * 1024:(q + 1) * 1024],
                                    reg[:, q * 1024:(q + 1) * 1024],
                                    EXP,
                                    bias=0.0,
                                    accum_out=partials[:, col + q:col + q + 1],
                                )
                        else:
                            nc.scalar.activation(
                                reg[:],
                                reg[:],
                                EXP,
                                bias=0.0,
                                accum_out=partials[:, col:col + 1],
                            )

            # P1: img rows x txt[:n]  (B1)
            do_pass(0, st_img, mov_t1, P1_HALVES, 0)
            nc.scalar.dma_start(st_txt[:], stat_txt[:])
            # ship B1's per-partition exp sums; the host does the final
            # partition+core reduction for the col sums.  Runs on the gpsimd
            # ring during P2, completely off the critical path.
            nc.gpsimd.dma_start(out_acc[:], acc[:])

            # P2: img rows x txt[n:]  (B2)
            do_pass(1, st_img, mov_t2, PW_HALVES, 8)

            # P3: txt rows x img[n:]  (B3 transposed)
            do_pass(2, st_txt, mov_i2, PW_HALVES, 16)

            # issued by the scalar engine itself right after its last
            # accumulator read: no cross-engine hop on the critical tail
            nc.scalar.dma_start(out_rows[:], partials[:])

    nc.compile()
    return nc


def _get_nc():
    if "nc" not in _CACHE:
        _CACHE["nc"] = _build()
    return _CACHE["nc"]


def _prep_inputs(image_features, text_features, logit_scale):
    img = np.asarray(image_features, dtype=np.float32)
    txt = np.asarray(text_features, dtype=np.float32)
    scale = float(np.asarray(logit_scale, dtype=np.float32))
    sf = np.float32(np.sqrt(scale))  # folded into BOTH operands

    qimg = np.asarray(img * sf, dtype=np.float32).astype(FP8NP)
    qtxt = np.asarray(txt * sf, dtype=np.float32).astype(FP8NP)

    def mov_layout(q):
        # [p, k, c] = q[c, k*128 + p]
        a = np.ascontiguousarray(q.T).reshape(KC, 128, HN)
        return np.ascontiguousarray(a.transpose(1, 0, 2))

    def stat_layout(q, c):
        # [p, k, m] = q[c*R + m, k*128 + p]
        a = np.ascontiguousarray(q[c * R:(c + 1) * R].T).reshape(KC, 128, R)
        return np.ascontiguousarray(a.transpose(1, 0, 2))

    mov_t1 = mov_layout(qtxt[:N])
    mov_t2 = mov_layout(qtxt[N:])
    mov_i2 = mov_layout(qimg[N:])
    in_maps = [
        {
            "stat_img": stat_layout(qimg, c),
            "stat_txt": stat_layout(qtxt, c),
            "mov_t1": mov_t1,
            "mov_t2": mov_t2,
            "mov_i2": mov_i2,
        }
        for c in range(C)
    ]
    # diagonal logits (same for both CE terms): scale * <img_r, txt_r>
    diag = scale * np.sum(
        img[:N].astype(np.float64) * txt[:N].astype(np.float64), axis=1
    )
    return in_maps, diag


def _finish(results, diag):
    P = np.stack([results[c]["out_rows"] for c in range(C)]).astype(np.float64)
    # col sums of exp(B1): reduce the per-core [128, 4096] bf16 partial sums
    # over partitions and cores in f64
    colp = (
        np.stack([results[c]["out_acc"] for c in range(C)])
        .astype(np.float64)
        .sum(axis=(0, 1))
    )
    # [C, 128, MB] row sums; partial col layout: pbase + m*2 + half
    s_img = (
        P[:, :, 0:8].reshape(C, 128, MB, 2).sum(-1)
        + P[:, :, 8:16].reshape(C, 128, MB, 2).sum(-1)
    )
    s_txt = np.empty_like(s_img)
    s_txt[:, :, 0:3] = P[:, :, 16:22].reshape(C, 128, 3, 2).sum(-1)
    s_txt[:, :, 3] = P[:, :, 22:25].sum(-1)
    # global row for (c, p, m): c*R + m*128 + p
    rows = (
        np.arange(C)[:, None, None] * R
        + np.arange(MB)[None, None, :] * 128
        + np.arange(128)[None, :, None]
    )
    s_txt = s_txt + colp[rows]
    d = diag[rows]
    ce_img = np.mean(np.log(s_img) - d)
    ce_txt = np.mean(np.log(s_txt) - d)
    return np.float32((ce_img + ce_txt) / 2.0)


def kernel(image_features, text_features, logit_scale):
    nc = _get_nc()
    in_maps, diag = _prep_inputs(image_features, text_features, logit_scale)
    res = run_bass_kernel_spmd(nc, in_maps, list(range(C)))
    return _finish(res.results, diag)


if __name__ == "__main__":
    rng = np.random.default_rng(0)
    img = rng.standard_normal((TWO_N, D), dtype=np.float32)
    txt = rng.standard_normal((TWO_N, D), dtype=np.float32)
    img /= np.linalg.norm(img, axis=-1, keepdims=True)
    txt /= np.linalg.norm(txt, axis=-1, keepdims=True)
    print(kernel(img, txt, np.float32(100.0)))
